# revision 31
# baseline (speedup 1.0000x reference)
"""Trainium2 Bass kernel for nn_FraudDetectionModel (temporal encoder + 2-layer
GAT + classifier). Self-contained: hardcodes shapes, shards across 8 cores.

Single device program: temporal encoder + GAT1 per destination-sharded edge
chunks; GAT1 node outputs written to a local HBM pair-row table, AllGathered
across the 8 cores, and per-edge source rows fetched on-device with GPSIMD
dma_gather (int16 A/B row-range split + flag-mask merge); GAT2 segment softmax
and the classifier finish in the same program. Only the [N,1] probabilities
leave the device.
"""
import numpy as np
import ml_dtypes

bf16 = ml_dtypes.bfloat16
NEG = -1.0e9


class _Cfg:
    def __init__(self, n=100000, e=1600000, ncore=8):
        self.N, self.E, self.NCORE = n, e, ncore
        self.T, self.F = 50, 10
        self.H1, self.C1, self.LAT = 4, 32, 64
        self.L = n // ncore
        self.TILES = (self.L + 127) // 128
        self.LP = self.TILES * 128
        self.QUADS = self.LP // 4                 # local quad rows (4 nodes/row)
        self.NR4 = ncore * self.QUADS             # gathered quad rows
        # temporal K-tiling
        self.TQ = [11, 11, 11, 11, 6]
        self.TQOFF = [0, 11, 22, 33, 44]
        self.KQ = [121, 121, 121, 121, 66]
        self.QW = [704, 704, 704, 704, 384]       # matmul cols per q (64*tq)


CFG = _Cfg()


# ======================================================================
# host prep: graph
# ======================================================================
def _prep_graph(cfg, edge_index):
    src = edge_index[0].astype(np.int64)
    dst = edge_index[1].astype(np.int64)
    loops = np.arange(cfg.N, dtype=np.int64)
    src = np.concatenate([src, loops])
    dst = np.concatenate([dst, loops])
    core = dst // cfg.L
    percore, invs = [], []
    for c in range(cfg.NCORE):
        m = core == c
        es, ed = src[m], dst[m] - c * cfg.L
        deg = np.bincount(ed, minlength=cfg.L)
        perm = np.argsort(deg, kind="stable")
        inv = np.empty(cfg.L, np.int64)
        inv[perm] = np.arange(cfg.L)
        degp = np.zeros(cfg.LP, np.int64)
        degp[:cfg.L] = deg[perm]
        pos = inv[ed]
        order = np.argsort(pos, kind="stable")
        percore.append((es[order], pos[order], degp, perm))
        invs.append(inv)
    D = np.ones(cfg.TILES, np.int64)
    for es, pos, degp, perm in percore:
        D = np.maximum(D, degp.reshape(cfg.TILES, 128).max(1))
    return percore, invs, D


def _chunk_sched(cfg, D, cap_slots=64, max_tiles=6):
    cap_slots = max(cap_slots, int(D.max()))
    chunks, t = [], 0
    while t < cfg.TILES:
        nt = 1
        while (t + nt < cfg.TILES and nt < max_tiles
               and int(max(D[t:t + nt + 1])) * (nt + 1) <= cap_slots):
            nt += 1
        chunks.append((t, nt, int(max(D[t:t + nt]))))
        t += nt
    offs = np.concatenate([[0], np.cumsum([nt * dc for _, nt, dc in chunks])])
    return chunks, offs.astype(np.int64)


def _edge_layout(cfg, percore_c, chunks, offs):
    """Flat slot index (lane*S + col) per edge for one core."""
    es, pos, degp, perm = percore_c
    S = int(offs[-1])
    start = np.concatenate([[0], np.cumsum(degp)])[:-1]
    rr = np.arange(len(pos)) - start[pos]
    tile_of = pos // 128
    lane = pos % 128
    tile2col = np.zeros(cfg.TILES, np.int64)
    for ci, (t0, nt, dc) in enumerate(chunks):
        tile2col[t0:t0 + nt] = offs[ci] + np.arange(nt) * dc
    col = tile2col[tile_of] + rr
    return lane * S + col, S, tile2col


# ======================================================================
# host prep: per-edge grids
# ======================================================================
XES = 14                 # xe slot stride: [x(10), a1s_h0..h3]


def _xe_grid(cfg, x, a1s, percore_c, flat, S, tile2col):
    """xe [128, S*14] bf16: [x(10), a1s_h0..h3]; pads: a1s cols = NEG.
    Dummy slot 0 of zero-degree lanes = this core's perm-slot-0 node."""
    es, pos, degp, perm = percore_c
    g = np.zeros((128 * S, XES), np.float32)
    g[:, cfg.F:cfg.F + cfg.H1] = NEG
    g[flat, :cfg.F] = x[es]
    g[flat, cfg.F:cfg.F + cfg.H1] = a1s[es]
    dummy = np.nonzero(degp == 0)[0]
    if len(dummy):
        lanes, tiles = dummy % 128, dummy // 128
        dflat = lanes * S + tile2col[tiles]
        n0 = perm[0]
        g[dflat, :cfg.F] = x[n0]
        g[dflat, cfg.F:cfg.F + cfg.H1] = a1s[n0]
    return g.reshape(128, S * XES).astype(bf16)


GMAX = 8                 # gather group width (<=1024 idxs per dma_gather)


def _wrap16_chunks(lin2d, chunks, offs):
    """lin2d [128, S] slot-major wrap per <=GMAX-col group -> [16, 8*S] int16."""
    S = lin2d.shape[1]
    out = np.zeros((16, 8 * S), np.int16)
    for ci, (t0, nt, dc) in enumerate(chunks):
        o0, o1 = int(offs[ci]), int(offs[ci + 1])
        for g0 in range(o0, o1, GMAX):
            g1 = min(g0 + GMAX, o1)
            lin = lin2d[:, g0:g1].T.reshape(-1)   # i = (col-g0)*128 + lane
            n = len(lin)
            w = np.zeros((16, n // 16), np.int16)
            w[np.arange(n) % 16, np.arange(n) // 16] = lin
            out[:, g0 * 8:g1 * 8] = w
    return out


def _gat2_planes(cfg, percore_c, invs, c, flat, S, tile2col):
    """Gather planes for GAT2: idx [16, 8S] i16 (quad rows), flags [128, S]."""
    es, pos, degp, perm = percore_c
    owner = es // cfg.L
    r = np.empty_like(es)
    for oc in range(cfg.NCORE):
        m = owner == oc
        if m.any():
            r[m] = invs[oc][es[m] - oc * cfg.L]
    grow = owner * cfg.LP + r

    idx = np.zeros((128 * S,), np.int64)
    flags = np.full((128 * S,), 9.0, np.float32)
    idx[flat] = grow // 4
    flags[flat] = grow % 4

    dummy = np.nonzero(degp == 0)[0]
    if len(dummy):
        lanes, tiles = dummy % 128, dummy // 128
        dflat = lanes * S + tile2col[tiles]
        dgrow = c * cfg.LP + 0                    # own perm-slot-0 node
        idx[dflat] = dgrow // 4
        flags[dflat] = dgrow % 4

    return (_wrap16_chunks(idx.reshape(128, S), *_cho(cfg)),
            flags.reshape(128, S).astype(bf16))


_CHO = None


def _cho(cfg):
    return _CHO


# ======================================================================
# host prep: packing
# ======================================================================
def _pack_td(cfg, td, perm, c):
    """Tight pack: tdA [TILES, 4, 121, 128] (q0..3), tdB [TILES, 66, 128]."""
    tdp = np.zeros((cfg.LP, cfg.T, cfg.F), np.float32)
    tdp[:cfg.L] = td[c * cfg.L + perm]
    nodes = tdp.reshape(cfg.TILES, 128, cfg.T, cfg.F)
    outA = np.zeros((cfg.TILES, 4, 121, 128), np.float32)
    outB = np.zeros((cfg.TILES, 66, 128), np.float32)
    for q in range(5):
        tq, kq = cfg.TQ[q], cfg.KQ[q]
        blk = nodes[:, :, cfg.TQOFF[q]:cfg.TQOFF[q] + tq, :]
        r = np.zeros((cfg.TILES, tq, 11, 128), np.float32)
        r[:, :, :cfg.F] = blk.transpose(0, 2, 3, 1)
        r[:, :, cfg.F] = 1.0
        if q < 4:
            outA[:, q] = r.reshape(cfg.TILES, kq, 128)
        else:
            outB[:] = r.reshape(cfg.TILES, kq, 128)
    return (np.ascontiguousarray(outA).astype(bf16),
            np.ascontiguousarray(outB).astype(bf16))


def _block_diag(b, n):
    r, c = b.shape
    out = np.zeros((n * r, n * c), np.float32)
    for i in range(n):
        out[i * r:(i + 1) * r, i * c:(i + 1) * c] = b
    return out


def _prep_weights(cfg, w):
    F, H1, C1, LAT = cfg.F, cfg.H1, cfg.C1, cfg.LAT
    gW1 = w["gW1"].astype(np.float32)
    A1s = (gW1.reshape(F, H1, C1) * w["ga1_src"][None]).sum(-1)
    A1d = (gW1.reshape(F, H1, C1) * w["ga1_dst"][None]).sum(-1)
    gW2 = w["gW2"].astype(np.float32)
    A2s = (gW2.reshape(H1 * C1, 1, LAT) * w["ga2_src"][None]).sum(-1)[:, 0]
    A2d = (gW2.reshape(H1 * C1, 1, LAT) * w["ga2_dst"][None]).sum(-1)[:, 0]

    con = {}
    W1b = np.concatenate([w["tW1"], w["tb1"][None]], 0).astype(np.float32)
    con["rhs_mm1f"] = _block_diag(W1b, 11)                    # [121, 704]
    con["rhs_mm1p"] = _block_diag(W1b, 6)                     # [66, 384]

    rhs_g1 = np.zeros((4 * F, H1 * C1), np.float32)
    for h in range(H1):
        rhs_g1[h * F:(h + 1) * F, h * C1:(h + 1) * C1] = gW1[:, h * C1:(h + 1) * C1]
    con["rhs_g1"] = rhs_g1                                    # [40, 128] f32
    con["gb1bc"] = np.tile(w["gb1"][None], (128, 1))          # [128, 128] f32

    gw2ext = np.zeros((H1 * C1, 68), np.float32)
    gw2ext[:, :LAT] = gW2
    gw2ext[:, LAT] = A2s
    gw2ext[:, LAT + 1] = A2d
    con["gw2ext"] = gw2ext                                    # [128, 68] f32

    ones = np.ones(H1 * C1, np.float32)
    adj2 = float(-(ones @ A2s) - (ones @ A2d))
    g2shift = ones @ gW2
    cW1 = w["cW1"].astype(np.float32)
    con["cw1f"] = np.concatenate(
        [(w["tW2"] / 50.0) @ cW1[:LAT], cW1[LAT:]], 0)        # [128, 64] f32
    cb1p = (w["tb2"] @ cW1[:LAT]) + ((w["gb2"] - g2shift) @ cW1[LAT:]) + w["cb1"]
    con["cb1bc"] = np.tile(cb1p[None], (128, 1))              # [128, 64] f32
    con["cw2bc"] = np.tile(w["cW2"][:, 0][None], (128, 1))    # [128, 64] f32
    return con, adj2, float(w["cb2"][0]), A1s, A1d


# ======================================================================
# device builder: single program
# ======================================================================
def _mk(ap_tensor, offset, dims):
    from concourse.bass import AP
    return AP(ap_tensor, int(offset), [list(d) for d in dims])


def build_exec(cfg, chunks, offs, adj2, cb2v):
    import concourse.bacc as bacc
    import concourse.mybir as mybir
    import concourse.tile as tile
    from concourse.masks import make_identity

    fp32 = mybir.dt.float32
    b16 = mybir.dt.bfloat16
    i16 = mybir.dt.int16
    F, H1, TQ, KQ, QW = cfg.F, cfg.H1, cfg.TQ, cfg.KQ, cfg.QW
    S = int(offs[-1])
    WMAX = max(nt * dc for _, nt, dc in chunks)
    NTMAX = max(nt for _, nt, dc in chunks)

    nc = bacc.Bacc(None, target_bir_lowering=False, debug=False)
    tdA = nc.dram_tensor("tdA", [cfg.TILES * 4 * 121, 128], b16, kind="ExternalInput")
    tdB = nc.dram_tensor("tdB", [cfg.TILES * 66, 128], b16, kind="ExternalInput")
    xe = nc.dram_tensor("xe", [128, S * XES], b16, kind="ExternalInput")
    a1d_i = nc.dram_tensor("a1d_i", [128, cfg.TILES * 4], b16, kind="ExternalInput")
    idxq = nc.dram_tensor("idxq", [16, 8 * S], i16, kind="ExternalInput")
    flg = nc.dram_tensor("flg", [128, S], b16, kind="ExternalInput")
    c_mm1f = nc.dram_tensor("c_mm1f", [121, 704], b16, kind="ExternalInput")
    c_mm1p = nc.dram_tensor("c_mm1p", [66, 384], b16, kind="ExternalInput")
    c_g1 = nc.dram_tensor("c_g1", [40, 128], fp32, kind="ExternalInput")
    c_gb1 = nc.dram_tensor("c_gb1", [128, 128], fp32, kind="ExternalInput")
    c_g2e = nc.dram_tensor("c_g2e", [128, 68], fp32, kind="ExternalInput")
    c_cw1 = nc.dram_tensor("c_cw1", [128, 64], fp32, kind="ExternalInput")
    c_cb1 = nc.dram_tensor("c_cb1", [128, 64], fp32, kind="ExternalInput")
    c_cw2 = nc.dram_tensor("c_cw2", [128, 64], fp32, kind="ExternalInput")
    o_p = nc.dram_tensor("o_p", [128, cfg.TILES], fp32, kind="ExternalOutput")

    with tile.TileContext(nc) as tc:
        with (
            tc.tile_pool(name="dram", bufs=1, space="DRAM") as dram,
            tc.tile_pool(name="const", bufs=1) as cp,
            tc.tile_pool(name="per", bufs=1) as per,
            tc.tile_pool(name="tds", bufs=3) as tds,
            tc.tile_pool(name="qps", bufs=2, space="PSUM") as qps,
            tc.tile_pool(name="gps", bufs=2, space="PSUM") as gps,
            tc.tile_pool(name="ev", bufs=2) as ev,
            tc.tile_pool(name="xep", bufs=2) as xep,
            tc.tile_pool(name="ed", bufs=2) as ed,
            tc.tile_pool(name="big", bufs=1) as big,
            tc.tile_pool(name="tl", bufs=2) as tl,
            tc.tile_pool(name="gth", bufs=1) as gth,
            tc.tile_pool(name="idxp", bufs=2) as idxp,
            tc.tile_pool(name="mrg", bufs=1) as mrg,
        ):
            g2loc = dram.tile([cfg.QUADS, 384], b16)
            g2all = dram.tile([cfg.NR4, 384], b16)

            ident = cp.tile([128, 128], fp32)
            make_identity(nc, ident[:])
            mm1f = cp.tile([121, 704], b16)
            nc.sync.dma_start(out=mm1f[:], in_=c_mm1f[:])
            mm1p = cp.tile([66, 384], b16)
            nc.sync.dma_start(out=mm1p[:], in_=c_mm1p[:])
            g1W = cp.tile([40, 128], fp32)
            nc.sync.dma_start(out=g1W[:], in_=c_g1[:])
            gb1bc = cp.tile([128, 128], fp32)
            nc.sync.dma_start(out=gb1bc[:], in_=c_gb1[:])
            g2eW = cp.tile([128, 68], fp32)
            nc.sync.dma_start(out=g2eW[:], in_=c_g2e[:])
            cw1 = cp.tile([128, 64], fp32)
            nc.sync.dma_start(out=cw1[:], in_=c_cw1[:])
            cb1 = cp.tile([128, 64], fp32)
            nc.sync.dma_start(out=cb1[:], in_=c_cb1[:])
            cw2 = cp.tile([128, 64], fp32)
            nc.sync.dma_start(out=cw2[:], in_=c_cw2[:])
            adj_t = cp.tile([128, 1], fp32)
            nc.vector.memset(adj_t[:], float(adj2))
            cb2_t = cp.tile([128, 1], fp32)
            nc.vector.memset(cb2_t[:], float(cb2v))
            a1d_all = per.tile([128, cfg.TILES * 4], b16)
            nc.sync.dma_start(out=a1d_all[:], in_=a1d_i[:])
            flA = per.tile([128, S], b16)
            nc.sync.dma_start(out=flA[:], in_=flg[:])

            spre = per.tile([128, cfg.TILES * 64], fp32)
            a2d_all = per.tile([128, cfg.TILES], fp32)
            prob = per.tile([128, cfg.TILES], fp32)

            # ---- GAT1 per chunk -> g2loc pair rows ----
            for ci, (t0, nt, dc) in enumerate(chunks):
                W = nt * dc
                xeb = xep.tile([128, WMAX * XES], b16, tag="xeb")
                nc.sync.dma_start(
                    out=xeb[:, 0:W * XES],
                    in_=xe[:, int(offs[ci]) * XES:(int(offs[ci]) + W) * XES])
                # esum layout (h, t, j): xe a1s col + a1d_dst
                esum = ed.tile([128, WMAX * 4], b16, tag="esum")
                for h in range(4):
                    nc.vector.tensor_tensor(
                        out=esum[:, h * W:(h + 1) * W],
                        in0=_mk(xeb.tensor, F + h,
                                [xeb[:].ap[0], [XES * dc, nt], [XES, dc]]),
                        in1=_mk(a1d_all.tensor, 4 * t0 + h,
                                [a1d_all[:].ap[0], [4, nt], [0, dc]]),
                        op=mybir.AluOpType.add)
                lr = ed.tile([128, WMAX * 4], b16, tag="lr")
                nc.vector.scalar_tensor_tensor(
                    out=lr[:, 0:W * 4], in0=esum[:, 0:W * 4], scalar=0.2,
                    in1=esum[:, 0:W * 4],
                    op0=mybir.AluOpType.mult, op1=mybir.AluOpType.max)
                wv = ed.tile([128, WMAX * 4], b16, tag="wv")
                nc.scalar.activation(out=wv[:, 0:W * 4], in_=lr[:, 0:W * 4],
                                     func=mybir.ActivationFunctionType.Exp)
                den = ed.tile([128, NTMAX * 4], fp32, tag="den")
                nc.vector.tensor_reduce(
                    out=den[:, 0:nt * 4],
                    in_=_mk(wv.tensor, 0, [wv[:].ap[0], [dc, 4 * nt], [1, dc]]),
                    axis=mybir.AxisListType.X, op=mybir.AluOpType.add)
                rec = ed.tile([128, NTMAX * 4], fp32, tag="rec")
                nc.vector.reciprocal(out=rec[:, 0:nt * 4], in_=den[:, 0:nt * 4])
                tmpx = ed.tile([128, WMAX * 10], b16, tag="tmpx")
                xaggr = ed.tile([128, NTMAX * 40], fp32, tag="xaggr")
                for h in range(4):
                    nc.vector.tensor_tensor(
                        out=tmpx[:, 0:W * 10],
                        in0=_mk(xeb.tensor, 0,
                                [xeb[:].ap[0], [XES * dc, nt], [XES, dc], [1, 10]]),
                        in1=_mk(wv.tensor, h * W,
                                [wv[:].ap[0], [dc, nt], [1, dc], [0, 10]]),
                        op=mybir.AluOpType.mult)
                    nc.vector.tensor_reduce(
                        out=_mk(xaggr.tensor, 10 * h,
                                [xaggr[:].ap[0], [40, nt], [1, 10]]),
                        in_=_mk(tmpx.tensor, 0,
                                [tmpx[:].ap[0], [10 * dc, nt], [1, 10], [10, dc]]),
                        axis=mybir.AxisListType.X, op=mybir.AluOpType.add)
                xagg = ed.tile([128, NTMAX * 40], fp32, tag="xagg")
                nc.vector.tensor_tensor(
                    out=xagg[:, 0:nt * 40],
                    in0=xaggr[:, 0:nt * 40],
                    in1=_mk(rec.tensor, 0,
                            [rec[:].ap[0], [1, nt], [nt, 4], [0, 10]]),
                    op=mybir.AluOpType.mult)
                for ti in range(nt):
                    t = t0 + ti
                    ps1 = gps.tile([128, 128], fp32, tag="g")
                    nc.tensor.transpose(out=ps1[0:40, :],
                                        in_=xagg[:, 40 * ti:40 * (ti + 1)],
                                        identity=ident[:])
                    stag = tl.tile([40, 128], fp32, tag="stag1")
                    nc.vector.tensor_copy(out=stag[:], in_=ps1[0:40, :])
                    out1 = gps.tile([128, 128], fp32, tag="g")
                    nc.tensor.matmul(out1[:], lhsT=stag[:], rhs=g1W[:],
                                     start=True, stop=True)
                    y1 = tl.tile([128, 128], fp32, tag="y1")
                    nc.vector.tensor_add(out=y1[:], in0=out1[:], in1=gb1bc[:])
                    mn = tl.tile([128, 128], fp32, tag="mn")
                    nc.vector.tensor_scalar_min(out=mn[:], in0=y1[:], scalar1=0.0)
                    ex = tl.tile([128, 128], fp32, tag="ex")
                    nc.scalar.activation(out=ex[:], in_=mn[:],
                                         func=mybir.ActivationFunctionType.Exp)
                    hs = tl.tile([128, 128], fp32, tag="hs")
                    nc.vector.scalar_tensor_tensor(
                        out=hs[:], in0=y1[:], scalar=0.0, in1=ex[:],
                        op0=mybir.AluOpType.max, op1=mybir.AluOpType.add)
                    ps2 = gps.tile([128, 128], fp32, tag="g")
                    nc.tensor.transpose(out=ps2[:], in_=hs[:], identity=ident[:])
                    hT = tl.tile([128, 128], fp32, tag="hT")
                    nc.vector.tensor_copy(out=hT[:], in_=ps2[:])
                    g2p = gps.tile([128, 128], fp32, tag="g")
                    nc.tensor.matmul(g2p[:, 0:68], lhsT=hT[:], rhs=g2eW[:],
                                     start=True, stop=True)
                    g2s = tl.tile([128, 68], b16, tag="g2s")
                    nc.vector.tensor_copy(out=g2s[:], in_=g2p[:, 0:68])
                    # quad-row write: row = 32t + p//4, quarter = p%4
                    nc.sync.dma_start(
                        out=_mk(g2loc.tensor, t * 32 * 384,
                                [[384, 32], [68, 4], [1, 68]]),
                        in_=g2s[:])
                    nc.scalar.activation(out=a2d_all[:, t:t + 1],
                                         in_=g2p[:, 65:66],
                                         func=mybir.ActivationFunctionType.Identity,
                                         bias=adj_t[:])

            # ---- halo exchange ----
            nc.gpsimd.collective_compute(
                "AllGather", mybir.AluOpType.bypass,
                replica_groups=[list(range(cfg.NCORE))],
                ins=[g2loc[:]], outs=[g2all[:]])

            # ---- temporal encoder: spre = sum_t relu(z_t) ----
            for t in range(cfg.TILES):
                st = tds.tile([128, 640], b16)
                nc.sync.dma_start(
                    out=_mk(st.tensor, 0, [[st[:].ap[0][0], 121], [128, 4], [1, 128]]),
                    in_=_mk(tdA, t * 4 * 121 * 128,
                            [[128, 121], [121 * 128, 4], [1, 128]]))
                nc.sync.dma_start(
                    out=_mk(st.tensor, 512, [[st[:].ap[0][0], 66], [1, 128]]),
                    in_=_mk(tdB, t * 66 * 128, [[128, 66], [1, 128]]))
                stg = ev.tile([128, 3200], b16, tag="evs")
                for q in range(5):
                    kq, w = KQ[q], QW[q]
                    rhs = mm1f if q < 4 else mm1p
                    ps = qps.tile([128, 704], fp32, tag="qtile")
                    lhsT = st[0:kq, 128 * q:128 * (q + 1)]
                    for c0 in range(0, w, 512):
                        c1 = min(c0 + 512, w)
                        nc.tensor.matmul(ps[:, c0:c1], lhsT=lhsT,
                                         rhs=rhs[0:kq, c0:c1], start=True, stop=True)
                    nc.scalar.activation(out=stg[:, 704 * q:704 * q + w],
                                         in_=ps[:, 0:w],
                                         func=mybir.ActivationFunctionType.Relu)
                nc.vector.tensor_reduce(
                    out=spre[:, 64 * t:64 * (t + 1)],
                    in_=_mk(stg.tensor, 0, [stg[:].ap[0], [1, 64], [64, 50]]),
                    axis=mybir.AxisListType.X, op=mybir.AluOpType.add)

            # ---- GAT2 per chunk (gather + segment softmax) + classifier ----
            for ci, (t0, nt, dc) in enumerate(chunks):
                W = nt * dc
                o0 = int(offs[ci])
                ia = idxp.tile([128, WMAX * 8], i16, tag="ia")
                nc.sync.dma_start(
                    out=ia[:, 0:W * 8],
                    in_=_mk(idxq, o0 * 8, [[0, 8], [8 * S, 16], [1, W * 8]]))
                gq = gth.tile([128, WMAX * 384], b16, tag="gq")
                for r0 in range(0, W, GMAX):
                    r1 = min(r0 + GMAX, W)
                    gw = r1 - r0
                    nc.gpsimd.dma_gather(
                        _mk(gq.tensor, r0 * 384, [gq[:].ap[0], [384, gw], [1, 384]]),
                        _mk(g2all.tensor, 0, [[384, cfg.NR4], [1, 384]]),
                        ia[:, r0 * 8:r1 * 8], gw * 128, gw * 128, 384)
                m = []
                for k in range(4):
                    mk_ = mrg.tile([128, WMAX], b16, tag=f"m{k}")
                    nc.vector.tensor_scalar(
                        out=mk_[:, 0:W], in0=flA[:, o0:o0 + W], scalar1=float(k),
                        scalar2=None, op0=mybir.AluOpType.is_equal)
                    m.append(mk_)
                vm = mrg.tile([128, WMAX], b16, tag="vm")
                nc.vector.tensor_scalar(
                    out=vm[:, 0:W], in0=flA[:, o0:o0 + W], scalar1=4.0,
                    scalar2=None, op0=mybir.AluOpType.is_lt)
                ge68 = mrg.tile([128, WMAX * 68], b16, tag="ge68")
                t1 = mrg.tile([128, WMAX * 68], b16, tag="t1")
                t2 = mrg.tile([128, WMAX * 68], b16, tag="t2")
                for q0 in (0, 2):
                    nc.vector.tensor_tensor(
                        out=t1[:, 0:W * 68],
                        in0=_mk(gq.tensor, 68 * q0, [gq[:].ap[0], [384, W], [1, 68]]),
                        in1=_mk(m[q0].tensor, 0, [m[q0][:].ap[0], [1, W], [0, 68]]),
                        op=mybir.AluOpType.mult)
                    nc.vector.tensor_tensor(
                        out=t2[:, 0:W * 68],
                        in0=_mk(gq.tensor, 68 * (q0 + 1),
                                [gq[:].ap[0], [384, W], [1, 68]]),
                        in1=_mk(m[q0 + 1].tensor, 0,
                                [m[q0 + 1][:].ap[0], [1, W], [0, 68]]),
                        op=mybir.AluOpType.mult)
                    if q0 == 0:
                        nc.vector.tensor_add(out=ge68[:, 0:W * 68],
                                             in0=t1[:, 0:W * 68], in1=t2[:, 0:W * 68])
                    else:
                        nc.vector.tensor_add(out=t1[:, 0:W * 68],
                                             in0=t1[:, 0:W * 68], in1=t2[:, 0:W * 68])
                        nc.vector.tensor_add(out=ge68[:, 0:W * 68],
                                             in0=ge68[:, 0:W * 68], in1=t1[:, 0:W * 68])
                # segment softmax over incoming edges
                es2 = ed.tile([128, WMAX], b16, tag="es2")
                nc.vector.tensor_tensor(
                    out=es2[:, 0:W],
                    in0=_mk(ge68.tensor, 64, [ge68[:].ap[0], [68, W]]),
                    in1=_mk(a2d_all.tensor, t0, [a2d_all[:].ap[0], [1, nt], [0, dc]]),
                    op=mybir.AluOpType.add)
                lr2 = ed.tile([128, WMAX], b16, tag="lr2")
                nc.vector.scalar_tensor_tensor(
                    out=lr2[:, 0:W], in0=es2[:, 0:W], scalar=0.2, in1=es2[:, 0:W],
                    op0=mybir.AluOpType.mult, op1=mybir.AluOpType.max)
                w2 = ed.tile([128, WMAX], b16, tag="w2")
                nc.scalar.activation(out=w2[:, 0:W], in_=lr2[:, 0:W],
                                     func=mybir.ActivationFunctionType.Exp)
                w2v = ed.tile([128, WMAX], b16, tag="w2v")
                nc.vector.tensor_tensor(out=w2v[:, 0:W], in0=w2[:, 0:W],
                                        in1=vm[:, 0:W], op=mybir.AluOpType.mult)
                den2 = ed.tile([128, NTMAX], fp32, tag="den2")
                nc.vector.tensor_reduce(
                    out=den2[:, 0:nt],
                    in_=_mk(w2v.tensor, 0, [w2v[:].ap[0], [dc, nt], [1, dc]]),
                    axis=mybir.AxisListType.X, op=mybir.AluOpType.add)
                rec2 = ed.tile([128, NTMAX], fp32, tag="rec2")
                nc.vector.reciprocal(out=rec2[:, 0:nt], in_=den2[:, 0:nt])
                w2n = ed.tile([128, WMAX], b16, tag="w2n")
                nc.vector.tensor_tensor(
                    out=w2n[:, 0:W], in0=w2v[:, 0:W],
                    in1=_mk(rec2.tensor, 0, [rec2[:].ap[0], [1, nt], [0, dc]]),
                    op=mybir.AluOpType.mult)
                tmp2 = big.tile([128, WMAX * 64], b16, tag="tmp2")
                nc.vector.tensor_tensor(
                    out=tmp2[:, 0:W * 64],
                    in0=_mk(ge68.tensor, 0, [ge68[:].ap[0], [68, W], [1, 64]]),
                    in1=_mk(w2n.tensor, 0, [w2n[:].ap[0], [1, W], [0, 64]]),
                    op=mybir.AluOpType.mult)
                out2 = big.tile([128, NTMAX * 64], fp32, tag="out2")
                nc.vector.tensor_reduce(
                    out=out2[:, 0:nt * 64],
                    in_=_mk(tmp2.tensor, 0,
                            [tmp2[:].ap[0], [64 * dc, nt], [1, 64], [64, dc]]),
                    axis=mybir.AxisListType.X, op=mybir.AluOpType.add)
                for ti in range(nt):
                    t = t0 + ti
                    ps1 = gps.tile([128, 128], fp32, tag="g")
                    nc.tensor.transpose(out=ps1[0:64, :],
                                        in_=spre[:, 64 * t:64 * (t + 1)],
                                        identity=ident[:])
                    stag = tl.tile([128, 128], fp32, tag="stag2")
                    nc.vector.tensor_copy(out=stag[0:64, :], in_=ps1[0:64, :])
                    ps2 = gps.tile([128, 128], fp32, tag="g")
                    nc.tensor.transpose(out=ps2[0:64, :],
                                        in_=out2[:, 64 * ti:64 * (ti + 1)],
                                        identity=ident[:])
                    nc.vector.tensor_copy(out=stag[64:128, :], in_=ps2[0:64, :])
                    z1 = gps.tile([128, 64], fp32, tag="g")
                    nc.tensor.matmul(z1[:], lhsT=stag[:], rhs=cw1[:],
                                     start=True, stop=True)
                    y = tl.tile([128, 64], fp32, tag="y")
                    nc.vector.tensor_add(out=y[:], in0=z1[:], in1=cb1[:])
                    nc.vector.tensor_scalar_max(out=y[:], in0=y[:], scalar1=0.0)
                    zt = tl.tile([128, 64], fp32, tag="zt")
                    nc.vector.tensor_tensor(out=zt[:], in0=y[:], in1=cw2[:],
                                            op=mybir.AluOpType.mult)
                    zz = tl.tile([128, 1], fp32, tag="zz")
                    nc.vector.tensor_reduce(out=zz[:], in_=zt[:],
                                            axis=mybir.AxisListType.X,
                                            op=mybir.AluOpType.add)
                    nc.scalar.activation(out=prob[:, t:t + 1], in_=zz[:],
                                         func=mybir.ActivationFunctionType.Sigmoid,
                                         bias=cb2_t[:])
            nc.sync.dma_start(out=o_p[:], in_=prob[:])
    nc.finalize()
    return nc


# ======================================================================
# top level
# ======================================================================
def _run(nc, in_maps, ncore):
    from concourse.bass_utils import run_bass_kernel_spmd
    return run_bass_kernel_spmd(nc, in_maps, core_ids=list(range(ncore))).results


def kernel(temporal_data, x, edge_index, tW1, tb1, tW2, tb2,
           gW1, ga1_src, ga1_dst, gb1, gW2, ga2_src, ga2_dst, gb2,
           cW1, cb1, cW2, cb2, _cfg=None, _runner=None):
    global _CHO
    cfg = _cfg or CFG
    x = np.asarray(x, np.float32)
    td = np.asarray(temporal_data, np.float32)
    w = dict(tW1=np.asarray(tW1, np.float32), tb1=np.asarray(tb1, np.float32),
             tW2=np.asarray(tW2, np.float32), tb2=np.asarray(tb2, np.float32),
             gW1=np.asarray(gW1, np.float32), ga1_src=np.asarray(ga1_src, np.float32),
             ga1_dst=np.asarray(ga1_dst, np.float32), gb1=np.asarray(gb1, np.float32),
             gW2=np.asarray(gW2, np.float32), ga2_src=np.asarray(ga2_src, np.float32),
             ga2_dst=np.asarray(ga2_dst, np.float32), gb2=np.asarray(gb2, np.float32),
             cW1=np.asarray(cW1, np.float32), cb1=np.asarray(cb1, np.float32),
             cW2=np.asarray(cW2, np.float32), cb2=np.asarray(cb2, np.float32))

    percore, invs, D = _prep_graph(cfg, edge_index)
    chunks, offs = _chunk_sched(cfg, D)
    _CHO = (chunks, offs)
    con, adj2, cb2v, A1s, A1d = _prep_weights(cfg, w)
    a1s_all = x @ A1s                       # [N, 4]
    a1d_vals = x @ A1d                      # [N, 4]

    ins = []
    for c in range(cfg.NCORE):
        flat, S, tile2col = _edge_layout(cfg, percore[c], chunks, offs)
        perm = percore[c][3]
        a1dg = np.zeros((cfg.LP, 4), np.float32)
        a1dg[:cfg.L] = a1d_vals[c * cfg.L + perm]
        ia, fl = _gat2_planes(cfg, percore[c], invs, c, flat, S, tile2col)
        tdA, tdB = _pack_td(cfg, td, perm, c)
        ins.append({
            "tdA": tdA.reshape(cfg.TILES * 4 * 121, 128),
            "tdB": tdB.reshape(cfg.TILES * 66, 128),
            "xe": _xe_grid(cfg, x, a1s_all, percore[c], flat, S, tile2col),
            "a1d_i": a1dg.reshape(cfg.TILES, 128, 4).transpose(1, 0, 2)
                         .reshape(128, cfg.TILES * 4).astype(bf16),
            "idxq": ia, "flg": fl,
            "c_mm1f": con["rhs_mm1f"].astype(bf16),
            "c_mm1p": con["rhs_mm1p"].astype(bf16),
            "c_g1": con["rhs_g1"].astype(np.float32),
            "c_gb1": con["gb1bc"].astype(np.float32),
            "c_g2e": con["gw2ext"].astype(np.float32),
            "c_cw1": con["cw1f"].astype(np.float32),
            "c_cb1": con["cb1bc"].astype(np.float32),
            "c_cw2": con["cw2bc"].astype(np.float32),
        })

    nc = build_exec(cfg, chunks, offs, adj2, cb2v)
    runner = _runner or _run
    res = runner(nc, ins, cfg.NCORE)

    out = np.zeros((cfg.N, 1), np.float32)
    for c in range(cfg.NCORE):
        p = np.asarray(res[c]["o_p"])           # [128, TILES] (lane, tile)
        pl = p.T.reshape(cfg.LP)                # perm position -> prob
        out[c * cfg.L:(c + 1) * cfg.L, 0] = pl[invs[c]]
    return out


# revision 35
# speedup vs baseline: 1.0025x; 1.0025x over previous
"""Trainium2 Bass kernel for nn_FraudDetectionModel (temporal encoder + 2-layer
GAT + classifier). Self-contained: hardcodes shapes, shards across 8 cores.

Single device program: temporal encoder + GAT1 per destination-sharded edge
chunks; GAT1 node outputs written to a local HBM quad-row table (4 nodes per
768B row), AllGathered across the 8 cores, and per-edge source rows fetched
on-device with GPSIMD dma_gather (single int16 stream, <=1024 idxs/call) and
flag-mask merged; GAT2 segment softmax and the classifier finish in the same
program. Only the [N,1] probabilities leave the device.
"""
import numpy as np
import ml_dtypes

bf16 = ml_dtypes.bfloat16
NEG = -1.0e9


class _Cfg:
    def __init__(self, n=100000, e=1600000, ncore=8):
        self.N, self.E, self.NCORE = n, e, ncore
        self.T, self.F = 50, 10
        self.H1, self.C1, self.LAT = 4, 32, 64
        self.L = n // ncore
        self.TILES = (self.L + 127) // 128
        self.LP = self.TILES * 128
        self.QUADS = self.LP // 4                 # local quad rows (4 nodes/row)
        self.NR4 = ncore * self.QUADS             # gathered quad rows
        # temporal K-tiling
        self.TQ = [11, 11, 11, 11, 6]
        self.TQOFF = [0, 11, 22, 33, 44]
        self.KQ = [121, 121, 121, 121, 66]
        self.QW = [704, 704, 704, 704, 384]       # matmul cols per q (64*tq)


CFG = _Cfg()


# ======================================================================
# host prep: graph
# ======================================================================
def _prep_graph(cfg, edge_index):
    src = edge_index[0].astype(np.int64)
    dst = edge_index[1].astype(np.int64)
    loops = np.arange(cfg.N, dtype=np.int64)
    src = np.concatenate([src, loops])
    dst = np.concatenate([dst, loops])
    core = dst // cfg.L
    percore, invs = [], []
    for c in range(cfg.NCORE):
        m = core == c
        es, ed = src[m], dst[m] - c * cfg.L
        deg = np.bincount(ed, minlength=cfg.L)
        perm = np.argsort(deg, kind="stable")
        inv = np.empty(cfg.L, np.int64)
        inv[perm] = np.arange(cfg.L)
        degp = np.zeros(cfg.LP, np.int64)
        degp[:cfg.L] = deg[perm]
        pos = inv[ed]
        order = np.argsort(pos, kind="stable")
        percore.append((es[order], pos[order], degp, perm))
        invs.append(inv)
    D = np.ones(cfg.TILES, np.int64)
    for es, pos, degp, perm in percore:
        D = np.maximum(D, degp.reshape(cfg.TILES, 128).max(1))
    return percore, invs, D


def _chunk_sched(cfg, D, cap_slots=64, max_tiles=6):
    cap_slots = max(cap_slots, int(D.max()))
    chunks, t = [], 0
    while t < cfg.TILES:
        nt = 1
        while (t + nt < cfg.TILES and nt < max_tiles
               and int(max(D[t:t + nt + 1])) * (nt + 1) <= cap_slots):
            nt += 1
        chunks.append((t, nt, int(max(D[t:t + nt]))))
        t += nt
    offs = np.concatenate([[0], np.cumsum([nt * dc for _, nt, dc in chunks])])
    return chunks, offs.astype(np.int64)


def _edge_layout(cfg, percore_c, chunks, offs):
    """Flat slot index (lane*S + col) per edge for one core."""
    es, pos, degp, perm = percore_c
    S = int(offs[-1])
    start = np.concatenate([[0], np.cumsum(degp)])[:-1]
    rr = np.arange(len(pos)) - start[pos]
    tile_of = pos // 128
    lane = pos % 128
    tile2col = np.zeros(cfg.TILES, np.int64)
    for ci, (t0, nt, dc) in enumerate(chunks):
        tile2col[t0:t0 + nt] = offs[ci] + np.arange(nt) * dc
    col = tile2col[tile_of] + rr
    return lane * S + col, S, tile2col


# ======================================================================
# host prep: per-edge grids
# ======================================================================
XES = 14                 # xe slot stride: [x(10), a1s_h0..h3]


def _xe_grid(cfg, x, a1s, percore_c, flat, S, tile2col):
    """xe [128, S*14] bf16: [x(10), a1s_h0..h3]; pads: a1s cols = NEG.
    Dummy slot 0 of zero-degree lanes = this core's perm-slot-0 node."""
    es, pos, degp, perm = percore_c
    g = np.zeros((128 * S, XES), np.float32)
    g[:, cfg.F:cfg.F + cfg.H1] = NEG
    g[flat, :cfg.F] = x[es]
    g[flat, cfg.F:cfg.F + cfg.H1] = a1s[es]
    dummy = np.nonzero(degp == 0)[0]
    if len(dummy):
        lanes, tiles = dummy % 128, dummy // 128
        dflat = lanes * S + tile2col[tiles]
        n0 = perm[0]
        g[dflat, :cfg.F] = x[n0]
        g[dflat, cfg.F:cfg.F + cfg.H1] = a1s[n0]
    return g.reshape(128, S * XES).astype(bf16)


GMAX = 8                 # gather group width (<=1024 idxs per dma_gather)


def _wrap16_chunks(lin2d, chunks, offs):
    """lin2d [128, S] slot-major wrap per <=GMAX-col group -> [16, 8*S] int16."""
    S = lin2d.shape[1]
    out = np.zeros((16, 8 * S), np.int16)
    for ci, (t0, nt, dc) in enumerate(chunks):
        o0, o1 = int(offs[ci]), int(offs[ci + 1])
        for g0 in range(o0, o1, GMAX):
            g1 = min(g0 + GMAX, o1)
            lin = lin2d[:, g0:g1].T.reshape(-1)   # i = (col-g0)*128 + lane
            n = len(lin)
            w = np.zeros((16, n // 16), np.int16)
            w[np.arange(n) % 16, np.arange(n) // 16] = lin
            out[:, g0 * 8:g1 * 8] = w
    return out


def _gat2_planes(cfg, percore_c, invs, c, flat, S, tile2col, chunks, offs):
    """Gather planes for GAT2: idx [16, 8S] i16 (quad rows), flags [128, S]."""
    es, pos, degp, perm = percore_c
    owner = es // cfg.L
    r = np.empty_like(es)
    for oc in range(cfg.NCORE):
        m = owner == oc
        if m.any():
            r[m] = invs[oc][es[m] - oc * cfg.L]
    grow = owner * cfg.LP + r

    idx = np.zeros((128 * S,), np.int64)
    flags = np.full((128 * S,), 9.0, np.float32)
    idx[flat] = grow // 4
    flags[flat] = grow % 4

    dummy = np.nonzero(degp == 0)[0]
    if len(dummy):
        lanes, tiles = dummy % 128, dummy // 128
        dflat = lanes * S + tile2col[tiles]
        dgrow = c * cfg.LP + 0                    # own perm-slot-0 node
        idx[dflat] = dgrow // 4
        flags[dflat] = dgrow % 4

    return (_wrap16_chunks(idx.reshape(128, S), chunks, offs),
            flags.reshape(128, S).astype(bf16))


# ======================================================================
# host prep: packing
# ======================================================================
def _pack_td(cfg, td, perm, c):
    """Tight pack: tdA [TILES, 4, 121, 128] (q0..3), tdB [TILES, 66, 128]."""
    tdp = np.zeros((cfg.LP, cfg.T, cfg.F), np.float32)
    tdp[:cfg.L] = td[c * cfg.L + perm]
    nodes = tdp.reshape(cfg.TILES, 128, cfg.T, cfg.F)
    outA = np.zeros((cfg.TILES, 4, 121, 128), np.float32)
    outB = np.zeros((cfg.TILES, 66, 128), np.float32)
    for q in range(5):
        tq, kq = cfg.TQ[q], cfg.KQ[q]
        blk = nodes[:, :, cfg.TQOFF[q]:cfg.TQOFF[q] + tq, :]
        r = np.zeros((cfg.TILES, tq, 11, 128), np.float32)
        r[:, :, :cfg.F] = blk.transpose(0, 2, 3, 1)
        r[:, :, cfg.F] = 1.0
        if q < 4:
            outA[:, q] = r.reshape(cfg.TILES, kq, 128)
        else:
            outB[:] = r.reshape(cfg.TILES, kq, 128)
    return (np.ascontiguousarray(outA).astype(bf16),
            np.ascontiguousarray(outB).astype(bf16))


def _block_diag(b, n):
    r, c = b.shape
    out = np.zeros((n * r, n * c), np.float32)
    for i in range(n):
        out[i * r:(i + 1) * r, i * c:(i + 1) * c] = b
    return out


def _prep_weights(cfg, w):
    F, H1, C1, LAT = cfg.F, cfg.H1, cfg.C1, cfg.LAT
    gW1 = w["gW1"].astype(np.float32)
    A1s = (gW1.reshape(F, H1, C1) * w["ga1_src"][None]).sum(-1)
    A1d = (gW1.reshape(F, H1, C1) * w["ga1_dst"][None]).sum(-1)
    gW2 = w["gW2"].astype(np.float32)
    A2s = (gW2.reshape(H1 * C1, 1, LAT) * w["ga2_src"][None]).sum(-1)[:, 0]
    A2d = (gW2.reshape(H1 * C1, 1, LAT) * w["ga2_dst"][None]).sum(-1)[:, 0]

    con = {}
    W1b = np.concatenate([w["tW1"], w["tb1"][None]], 0).astype(np.float32)
    con["rhs_mm1f"] = _block_diag(W1b, 11)                    # [121, 704]
    con["rhs_mm1p"] = _block_diag(W1b, 6)                     # [66, 384]

    rhs_g1 = np.zeros((4 * F, H1 * C1), np.float32)
    for h in range(H1):
        rhs_g1[h * F:(h + 1) * F, h * C1:(h + 1) * C1] = gW1[:, h * C1:(h + 1) * C1]
    con["rhs_g1"] = rhs_g1                                    # [40, 128] f32
    con["gb1bc"] = np.tile(w["gb1"][None], (128, 1))          # [128, 128] f32

    gw2ext = np.zeros((H1 * C1, 68), np.float32)
    gw2ext[:, :LAT] = gW2
    gw2ext[:, LAT] = A2s
    gw2ext[:, LAT + 1] = A2d
    con["gw2ext"] = gw2ext                                    # [128, 68] f32

    ones = np.ones(H1 * C1, np.float32)
    adj2 = float(-(ones @ A2s) - (ones @ A2d))
    g2shift = ones @ gW2
    cW1 = w["cW1"].astype(np.float32)
    con["cw1f"] = np.concatenate(
        [(w["tW2"] / 50.0) @ cW1[:LAT], cW1[LAT:]], 0)        # [128, 64] f32
    cb1p = (w["tb2"] @ cW1[:LAT]) + ((w["gb2"] - g2shift) @ cW1[LAT:]) + w["cb1"]
    con["cb1bc"] = np.tile(cb1p[None], (128, 1))              # [128, 64] f32
    con["cw2bc"] = np.tile(w["cW2"][:, 0][None], (128, 1))    # [128, 64] f32
    return con, adj2, float(w["cb2"][0]), A1s, A1d


# ======================================================================
# device builder: single program
# ======================================================================
def _mk(ap_tensor, offset, dims):
    from concourse.bass import AP
    return AP(ap_tensor, int(offset), [list(d) for d in dims])


def build_exec(cfg, chunks, offs, adj2, cb2v):
    import concourse.bacc as bacc
    import concourse.mybir as mybir
    import concourse.tile as tile
    from concourse.masks import make_identity

    fp32 = mybir.dt.float32
    b16 = mybir.dt.bfloat16
    i16 = mybir.dt.int16
    F, H1, TQ, KQ, QW = cfg.F, cfg.H1, cfg.TQ, cfg.KQ, cfg.QW
    S = int(offs[-1])
    WMAX = max(nt * dc for _, nt, dc in chunks)
    NTMAX = max(nt for _, nt, dc in chunks)

    nc = bacc.Bacc(None, target_bir_lowering=False, debug=False)
    tdA = nc.dram_tensor("tdA", [cfg.TILES * 4 * 121, 128], b16, kind="ExternalInput")
    tdB = nc.dram_tensor("tdB", [cfg.TILES * 66, 128], b16, kind="ExternalInput")
    xe = nc.dram_tensor("xe", [128, S * XES], b16, kind="ExternalInput")
    a1d_i = nc.dram_tensor("a1d_i", [128, cfg.TILES * 4], b16, kind="ExternalInput")
    idxq = nc.dram_tensor("idxq", [16, 8 * S], i16, kind="ExternalInput")
    flg = nc.dram_tensor("flg", [128, S], b16, kind="ExternalInput")
    c_mm1f = nc.dram_tensor("c_mm1f", [121, 704], b16, kind="ExternalInput")
    c_mm1p = nc.dram_tensor("c_mm1p", [66, 384], b16, kind="ExternalInput")
    c_g1 = nc.dram_tensor("c_g1", [40, 128], fp32, kind="ExternalInput")
    c_gb1 = nc.dram_tensor("c_gb1", [128, 128], fp32, kind="ExternalInput")
    c_g2e = nc.dram_tensor("c_g2e", [128, 68], fp32, kind="ExternalInput")
    c_cw1 = nc.dram_tensor("c_cw1", [128, 64], fp32, kind="ExternalInput")
    c_cb1 = nc.dram_tensor("c_cb1", [128, 64], fp32, kind="ExternalInput")
    c_cw2 = nc.dram_tensor("c_cw2", [128, 64], fp32, kind="ExternalInput")
    o_p = nc.dram_tensor("o_p", [128, cfg.TILES], fp32, kind="ExternalOutput")

    with tile.TileContext(nc) as tc:
        with (
            tc.tile_pool(name="dram", bufs=1, space="DRAM") as dram,
            tc.tile_pool(name="const", bufs=1) as cp,
            tc.tile_pool(name="per", bufs=1) as per,
            tc.tile_pool(name="tds", bufs=3) as tds,
            tc.tile_pool(name="qps", bufs=2, space="PSUM") as qps,
            tc.tile_pool(name="gps", bufs=2, space="PSUM") as gps,
            tc.tile_pool(name="ev", bufs=2) as ev,
            tc.tile_pool(name="xep", bufs=2) as xep,
            tc.tile_pool(name="ed", bufs=2) as ed,
            tc.tile_pool(name="big", bufs=1) as big,
            tc.tile_pool(name="tl", bufs=2) as tl,
            tc.tile_pool(name="gth", bufs=1) as gth,
            tc.tile_pool(name="idxp", bufs=2) as idxp,
            tc.tile_pool(name="mrg", bufs=1) as mrg,
        ):
            g2loc = dram.tile([cfg.QUADS, 384], b16)
            g2all = dram.tile([cfg.NR4, 384], b16)

            ident = cp.tile([128, 128], fp32)
            make_identity(nc, ident[:])
            mm1f = cp.tile([121, 704], b16)
            nc.sync.dma_start(out=mm1f[:], in_=c_mm1f[:])
            mm1p = cp.tile([66, 384], b16)
            nc.sync.dma_start(out=mm1p[:], in_=c_mm1p[:])
            g1W = cp.tile([40, 128], fp32)
            nc.sync.dma_start(out=g1W[:], in_=c_g1[:])
            gb1bc = cp.tile([128, 128], fp32)
            nc.sync.dma_start(out=gb1bc[:], in_=c_gb1[:])
            g2eW = cp.tile([128, 68], fp32)
            nc.sync.dma_start(out=g2eW[:], in_=c_g2e[:])
            cw1 = cp.tile([128, 64], fp32)
            nc.sync.dma_start(out=cw1[:], in_=c_cw1[:])
            cb1 = cp.tile([128, 64], fp32)
            nc.sync.dma_start(out=cb1[:], in_=c_cb1[:])
            cw2 = cp.tile([128, 64], fp32)
            nc.sync.dma_start(out=cw2[:], in_=c_cw2[:])
            adj_t = cp.tile([128, 1], fp32)
            nc.vector.memset(adj_t[:], float(adj2))
            cb2_t = cp.tile([128, 1], fp32)
            nc.vector.memset(cb2_t[:], float(cb2v))
            a1d_all = per.tile([128, cfg.TILES * 4], b16)
            nc.sync.dma_start(out=a1d_all[:], in_=a1d_i[:])
            flA = per.tile([128, S], b16)
            nc.sync.dma_start(out=flA[:], in_=flg[:])

            spre = per.tile([128, cfg.TILES * 64], fp32)
            a2d_all = per.tile([128, cfg.TILES], fp32)
            prob = per.tile([128, cfg.TILES], fp32)

            # ---- GAT1 per chunk -> g2loc pair rows ----
            for ci, (t0, nt, dc) in enumerate(chunks):
                W = nt * dc
                xeb = xep.tile([128, WMAX * XES], b16, tag="xeb")
                nc.sync.dma_start(
                    out=xeb[:, 0:W * XES],
                    in_=xe[:, int(offs[ci]) * XES:(int(offs[ci]) + W) * XES])
                # esum layout (h, t, j): xe a1s col + a1d_dst
                esum = ed.tile([128, WMAX * 4], b16, tag="esum")
                for h in range(4):
                    nc.vector.tensor_tensor(
                        out=esum[:, h * W:(h + 1) * W],
                        in0=_mk(xeb.tensor, F + h,
                                [xeb[:].ap[0], [XES * dc, nt], [XES, dc]]),
                        in1=_mk(a1d_all.tensor, 4 * t0 + h,
                                [a1d_all[:].ap[0], [4, nt], [0, dc]]),
                        op=mybir.AluOpType.add)
                lr = ed.tile([128, WMAX * 4], b16, tag="lr")
                nc.vector.scalar_tensor_tensor(
                    out=lr[:, 0:W * 4], in0=esum[:, 0:W * 4], scalar=0.2,
                    in1=esum[:, 0:W * 4],
                    op0=mybir.AluOpType.mult, op1=mybir.AluOpType.max)
                wv = ed.tile([128, WMAX * 4], b16, tag="wv")
                nc.scalar.activation(out=wv[:, 0:W * 4], in_=lr[:, 0:W * 4],
                                     func=mybir.ActivationFunctionType.Exp)
                den = ed.tile([128, NTMAX * 4], fp32, tag="den")
                nc.vector.tensor_reduce(
                    out=den[:, 0:nt * 4],
                    in_=_mk(wv.tensor, 0, [wv[:].ap[0], [dc, 4 * nt], [1, dc]]),
                    axis=mybir.AxisListType.X, op=mybir.AluOpType.add)
                rec = ed.tile([128, NTMAX * 4], fp32, tag="rec")
                nc.vector.reciprocal(out=rec[:, 0:nt * 4], in_=den[:, 0:nt * 4])
                tmpx = ed.tile([128, WMAX * 10], b16, tag="tmpx")
                xaggr = ed.tile([128, NTMAX * 40], fp32, tag="xaggr")
                for h in range(4):
                    nc.vector.tensor_tensor(
                        out=tmpx[:, 0:W * 10],
                        in0=_mk(xeb.tensor, 0,
                                [xeb[:].ap[0], [XES * dc, nt], [XES, dc], [1, 10]]),
                        in1=_mk(wv.tensor, h * W,
                                [wv[:].ap[0], [dc, nt], [1, dc], [0, 10]]),
                        op=mybir.AluOpType.mult)
                    nc.vector.tensor_reduce(
                        out=_mk(xaggr.tensor, 10 * h,
                                [xaggr[:].ap[0], [40, nt], [1, 10]]),
                        in_=_mk(tmpx.tensor, 0,
                                [tmpx[:].ap[0], [10 * dc, nt], [1, 10], [10, dc]]),
                        axis=mybir.AxisListType.X, op=mybir.AluOpType.add)
                xagg = ed.tile([128, NTMAX * 40], fp32, tag="xagg")
                nc.vector.tensor_tensor(
                    out=xagg[:, 0:nt * 40],
                    in0=xaggr[:, 0:nt * 40],
                    in1=_mk(rec.tensor, 0,
                            [rec[:].ap[0], [1, nt], [nt, 4], [0, 10]]),
                    op=mybir.AluOpType.mult)
                for ti in range(nt):
                    t = t0 + ti
                    ps1 = gps.tile([128, 128], fp32, tag="g")
                    nc.tensor.transpose(out=ps1[0:40, :],
                                        in_=xagg[:, 40 * ti:40 * (ti + 1)],
                                        identity=ident[:])
                    stag = tl.tile([40, 128], fp32, tag="stag1")
                    nc.vector.tensor_copy(out=stag[:], in_=ps1[0:40, :])
                    out1 = gps.tile([128, 128], fp32, tag="g")
                    nc.tensor.matmul(out1[:], lhsT=stag[:], rhs=g1W[:],
                                     start=True, stop=True)
                    y1 = tl.tile([128, 128], fp32, tag="y1")
                    nc.vector.tensor_add(out=y1[:], in0=out1[:], in1=gb1bc[:])
                    mn = tl.tile([128, 128], fp32, tag="mn")
                    nc.vector.tensor_scalar_min(out=mn[:], in0=y1[:], scalar1=0.0)
                    ex = tl.tile([128, 128], fp32, tag="ex")
                    nc.scalar.activation(out=ex[:], in_=mn[:],
                                         func=mybir.ActivationFunctionType.Exp)
                    hs = tl.tile([128, 128], fp32, tag="hs")
                    nc.vector.scalar_tensor_tensor(
                        out=hs[:], in0=y1[:], scalar=0.0, in1=ex[:],
                        op0=mybir.AluOpType.max, op1=mybir.AluOpType.add)
                    ps2 = gps.tile([128, 128], fp32, tag="g")
                    nc.tensor.transpose(out=ps2[:], in_=hs[:], identity=ident[:])
                    hT = tl.tile([128, 128], fp32, tag="hT")
                    nc.vector.tensor_copy(out=hT[:], in_=ps2[:])
                    g2p = gps.tile([128, 128], fp32, tag="g")
                    nc.tensor.matmul(g2p[:, 0:68], lhsT=hT[:], rhs=g2eW[:],
                                     start=True, stop=True)
                    g2s = tl.tile([128, 68], b16, tag="g2s")
                    nc.vector.tensor_copy(out=g2s[:], in_=g2p[:, 0:68])
                    # quad-row write: row = 32t + p//4, quarter = p%4
                    nc.sync.dma_start(
                        out=_mk(g2loc.tensor, t * 32 * 384,
                                [[384, 32], [68, 4], [1, 68]]),
                        in_=g2s[:])
                    nc.scalar.activation(out=a2d_all[:, t:t + 1],
                                         in_=g2p[:, 65:66],
                                         func=mybir.ActivationFunctionType.Identity,
                                         bias=adj_t[:])

            # ---- halo exchange ----
            nc.gpsimd.collective_compute(
                "AllGather", mybir.AluOpType.bypass,
                replica_groups=[list(range(cfg.NCORE))],
                ins=[g2loc[:]], outs=[g2all[:]])

            # ---- temporal encoder: spre = sum_t relu(z_t) ----
            for t in range(cfg.TILES):
                st = tds.tile([128, 640], b16)
                nc.sync.dma_start(
                    out=_mk(st.tensor, 0, [[st[:].ap[0][0], 121], [128, 4], [1, 128]]),
                    in_=_mk(tdA, t * 4 * 121 * 128,
                            [[128, 121], [121 * 128, 4], [1, 128]]))
                nc.sync.dma_start(
                    out=_mk(st.tensor, 512, [[st[:].ap[0][0], 66], [1, 128]]),
                    in_=_mk(tdB, t * 66 * 128, [[128, 66], [1, 128]]))
                stg = ev.tile([128, 3200], b16, tag="evs")
                for q in range(5):
                    kq, w = KQ[q], QW[q]
                    rhs = mm1f if q < 4 else mm1p
                    ps = qps.tile([128, 704], fp32, tag="qtile")
                    lhsT = st[0:kq, 128 * q:128 * (q + 1)]
                    for c0 in range(0, w, 512):
                        c1 = min(c0 + 512, w)
                        nc.tensor.matmul(ps[:, c0:c1], lhsT=lhsT,
                                         rhs=rhs[0:kq, c0:c1], start=True, stop=True)
                    nc.scalar.activation(out=stg[:, 704 * q:704 * q + w],
                                         in_=ps[:, 0:w],
                                         func=mybir.ActivationFunctionType.Relu)
                nc.vector.tensor_reduce(
                    out=spre[:, 64 * t:64 * (t + 1)],
                    in_=_mk(stg.tensor, 0, [stg[:].ap[0], [1, 64], [64, 50]]),
                    axis=mybir.AxisListType.X, op=mybir.AluOpType.add)

            # ---- GAT2 per chunk (gather + segment softmax) + classifier ----
            for ci, (t0, nt, dc) in enumerate(chunks):
                W = nt * dc
                o0 = int(offs[ci])
                ia = idxp.tile([128, WMAX * 8], i16, tag="ia")
                nc.sync.dma_start(
                    out=ia[:, 0:W * 8],
                    in_=_mk(idxq, o0 * 8, [[0, 8], [8 * S, 16], [1, W * 8]]))
                gq = gth.tile([128, WMAX * 384], b16, tag="gq")
                for r0 in range(0, W, GMAX):
                    r1 = min(r0 + GMAX, W)
                    gw = r1 - r0
                    nc.gpsimd.dma_gather(
                        _mk(gq.tensor, r0 * 384, [gq[:].ap[0], [384, gw], [1, 384]]),
                        _mk(g2all.tensor, 0, [[384, cfg.NR4], [1, 384]]),
                        ia[:, r0 * 8:r1 * 8], gw * 128, gw * 128, 384)
                m = []
                for k in range(4):
                    mk_ = mrg.tile([128, WMAX], b16, tag=f"m{k}")
                    nc.vector.tensor_scalar(
                        out=mk_[:, 0:W], in0=flA[:, o0:o0 + W], scalar1=float(k),
                        scalar2=None, op0=mybir.AluOpType.is_equal)
                    m.append(mk_)
                vm = mrg.tile([128, WMAX], b16, tag="vm")
                nc.vector.tensor_scalar(
                    out=vm[:, 0:W], in0=flA[:, o0:o0 + W], scalar1=4.0,
                    scalar2=None, op0=mybir.AluOpType.is_lt)
                ge68 = mrg.tile([128, WMAX * 68], b16, tag="ge68")
                t1 = mrg.tile([128, WMAX * 68], b16, tag="t1")
                t2 = mrg.tile([128, WMAX * 68], b16, tag="t2")
                for q0 in (0, 2):
                    nc.vector.tensor_tensor(
                        out=t1[:, 0:W * 68],
                        in0=_mk(gq.tensor, 68 * q0, [gq[:].ap[0], [384, W], [1, 68]]),
                        in1=_mk(m[q0].tensor, 0, [m[q0][:].ap[0], [1, W], [0, 68]]),
                        op=mybir.AluOpType.mult)
                    nc.vector.tensor_tensor(
                        out=t2[:, 0:W * 68],
                        in0=_mk(gq.tensor, 68 * (q0 + 1),
                                [gq[:].ap[0], [384, W], [1, 68]]),
                        in1=_mk(m[q0 + 1].tensor, 0,
                                [m[q0 + 1][:].ap[0], [1, W], [0, 68]]),
                        op=mybir.AluOpType.mult)
                    if q0 == 0:
                        nc.vector.tensor_add(out=ge68[:, 0:W * 68],
                                             in0=t1[:, 0:W * 68], in1=t2[:, 0:W * 68])
                    else:
                        nc.vector.tensor_add(out=t1[:, 0:W * 68],
                                             in0=t1[:, 0:W * 68], in1=t2[:, 0:W * 68])
                        nc.vector.tensor_add(out=ge68[:, 0:W * 68],
                                             in0=ge68[:, 0:W * 68], in1=t1[:, 0:W * 68])
                # segment softmax over incoming edges
                es2 = ed.tile([128, WMAX], b16, tag="es2")
                nc.vector.tensor_tensor(
                    out=es2[:, 0:W],
                    in0=_mk(ge68.tensor, 64, [ge68[:].ap[0], [68, W]]),
                    in1=_mk(a2d_all.tensor, t0, [a2d_all[:].ap[0], [1, nt], [0, dc]]),
                    op=mybir.AluOpType.add)
                lr2 = ed.tile([128, WMAX], b16, tag="lr2")
                nc.vector.scalar_tensor_tensor(
                    out=lr2[:, 0:W], in0=es2[:, 0:W], scalar=0.2, in1=es2[:, 0:W],
                    op0=mybir.AluOpType.mult, op1=mybir.AluOpType.max)
                w2 = ed.tile([128, WMAX], b16, tag="w2")
                nc.scalar.activation(out=w2[:, 0:W], in_=lr2[:, 0:W],
                                     func=mybir.ActivationFunctionType.Exp)
                w2v = ed.tile([128, WMAX], b16, tag="w2v")
                nc.vector.tensor_tensor(out=w2v[:, 0:W], in0=w2[:, 0:W],
                                        in1=vm[:, 0:W], op=mybir.AluOpType.mult)
                den2 = ed.tile([128, NTMAX], fp32, tag="den2")
                nc.vector.tensor_reduce(
                    out=den2[:, 0:nt],
                    in_=_mk(w2v.tensor, 0, [w2v[:].ap[0], [dc, nt], [1, dc]]),
                    axis=mybir.AxisListType.X, op=mybir.AluOpType.add)
                rec2 = ed.tile([128, NTMAX], fp32, tag="rec2")
                nc.vector.reciprocal(out=rec2[:, 0:nt], in_=den2[:, 0:nt])
                w2n = ed.tile([128, WMAX], b16, tag="w2n")
                nc.vector.tensor_tensor(
                    out=w2n[:, 0:W], in0=w2v[:, 0:W],
                    in1=_mk(rec2.tensor, 0, [rec2[:].ap[0], [1, nt], [0, dc]]),
                    op=mybir.AluOpType.mult)
                tmp2 = big.tile([128, WMAX * 64], b16, tag="tmp2")
                nc.vector.tensor_tensor(
                    out=tmp2[:, 0:W * 64],
                    in0=_mk(ge68.tensor, 0, [ge68[:].ap[0], [68, W], [1, 64]]),
                    in1=_mk(w2n.tensor, 0, [w2n[:].ap[0], [1, W], [0, 64]]),
                    op=mybir.AluOpType.mult)
                out2 = big.tile([128, NTMAX * 64], fp32, tag="out2")
                nc.vector.tensor_reduce(
                    out=out2[:, 0:nt * 64],
                    in_=_mk(tmp2.tensor, 0,
                            [tmp2[:].ap[0], [64 * dc, nt], [1, 64], [64, dc]]),
                    axis=mybir.AxisListType.X, op=mybir.AluOpType.add)
                for ti in range(nt):
                    t = t0 + ti
                    ps1 = gps.tile([128, 128], fp32, tag="g")
                    nc.tensor.transpose(out=ps1[0:64, :],
                                        in_=spre[:, 64 * t:64 * (t + 1)],
                                        identity=ident[:])
                    stag = tl.tile([128, 128], fp32, tag="stag2")
                    nc.vector.tensor_copy(out=stag[0:64, :], in_=ps1[0:64, :])
                    ps2 = gps.tile([128, 128], fp32, tag="g")
                    nc.tensor.transpose(out=ps2[0:64, :],
                                        in_=out2[:, 64 * ti:64 * (ti + 1)],
                                        identity=ident[:])
                    nc.vector.tensor_copy(out=stag[64:128, :], in_=ps2[0:64, :])
                    z1 = gps.tile([128, 64], fp32, tag="g")
                    nc.tensor.matmul(z1[:], lhsT=stag[:], rhs=cw1[:],
                                     start=True, stop=True)
                    y = tl.tile([128, 64], fp32, tag="y")
                    nc.vector.tensor_add(out=y[:], in0=z1[:], in1=cb1[:])
                    nc.vector.tensor_scalar_max(out=y[:], in0=y[:], scalar1=0.0)
                    zt = tl.tile([128, 64], fp32, tag="zt")
                    nc.vector.tensor_tensor(out=zt[:], in0=y[:], in1=cw2[:],
                                            op=mybir.AluOpType.mult)
                    zz = tl.tile([128, 1], fp32, tag="zz")
                    nc.vector.tensor_reduce(out=zz[:], in_=zt[:],
                                            axis=mybir.AxisListType.X,
                                            op=mybir.AluOpType.add)
                    nc.scalar.activation(out=prob[:, t:t + 1], in_=zz[:],
                                         func=mybir.ActivationFunctionType.Sigmoid,
                                         bias=cb2_t[:])
            nc.sync.dma_start(out=o_p[:], in_=prob[:])
    nc.finalize()
    return nc


# ======================================================================
# top level
# ======================================================================
def _run(nc, in_maps, ncore):
    from concourse.bass_utils import run_bass_kernel_spmd
    return run_bass_kernel_spmd(nc, in_maps, core_ids=list(range(ncore))).results


def kernel(temporal_data, x, edge_index, tW1, tb1, tW2, tb2,
           gW1, ga1_src, ga1_dst, gb1, gW2, ga2_src, ga2_dst, gb2,
           cW1, cb1, cW2, cb2, _cfg=None, _runner=None):
    cfg = _cfg or CFG
    x = np.asarray(x, np.float32)
    td = np.asarray(temporal_data, np.float32)
    w = dict(tW1=np.asarray(tW1, np.float32), tb1=np.asarray(tb1, np.float32),
             tW2=np.asarray(tW2, np.float32), tb2=np.asarray(tb2, np.float32),
             gW1=np.asarray(gW1, np.float32), ga1_src=np.asarray(ga1_src, np.float32),
             ga1_dst=np.asarray(ga1_dst, np.float32), gb1=np.asarray(gb1, np.float32),
             gW2=np.asarray(gW2, np.float32), ga2_src=np.asarray(ga2_src, np.float32),
             ga2_dst=np.asarray(ga2_dst, np.float32), gb2=np.asarray(gb2, np.float32),
             cW1=np.asarray(cW1, np.float32), cb1=np.asarray(cb1, np.float32),
             cW2=np.asarray(cW2, np.float32), cb2=np.asarray(cb2, np.float32))

    percore, invs, D = _prep_graph(cfg, edge_index)
    chunks, offs = _chunk_sched(cfg, D)
    con, adj2, cb2v, A1s, A1d = _prep_weights(cfg, w)
    a1s_all = x @ A1s                       # [N, 4]
    a1d_vals = x @ A1d                      # [N, 4]

    ins = []
    for c in range(cfg.NCORE):
        flat, S, tile2col = _edge_layout(cfg, percore[c], chunks, offs)
        perm = percore[c][3]
        a1dg = np.zeros((cfg.LP, 4), np.float32)
        a1dg[:cfg.L] = a1d_vals[c * cfg.L + perm]
        ia, fl = _gat2_planes(cfg, percore[c], invs, c, flat, S, tile2col,
                              chunks, offs)
        tdA, tdB = _pack_td(cfg, td, perm, c)
        ins.append({
            "tdA": tdA.reshape(cfg.TILES * 4 * 121, 128),
            "tdB": tdB.reshape(cfg.TILES * 66, 128),
            "xe": _xe_grid(cfg, x, a1s_all, percore[c], flat, S, tile2col),
            "a1d_i": a1dg.reshape(cfg.TILES, 128, 4).transpose(1, 0, 2)
                         .reshape(128, cfg.TILES * 4).astype(bf16),
            "idxq": ia, "flg": fl,
            "c_mm1f": con["rhs_mm1f"].astype(bf16),
            "c_mm1p": con["rhs_mm1p"].astype(bf16),
            "c_g1": con["rhs_g1"].astype(np.float32),
            "c_gb1": con["gb1bc"].astype(np.float32),
            "c_g2e": con["gw2ext"].astype(np.float32),
            "c_cw1": con["cw1f"].astype(np.float32),
            "c_cb1": con["cb1bc"].astype(np.float32),
            "c_cw2": con["cw2bc"].astype(np.float32),
        })

    nc = build_exec(cfg, chunks, offs, adj2, cb2v)
    runner = _runner or _run
    res = runner(nc, ins, cfg.NCORE)

    out = np.zeros((cfg.N, 1), np.float32)
    for c in range(cfg.NCORE):
        p = np.asarray(res[c]["o_p"])           # [128, TILES] (lane, tile)
        pl = p.T.reshape(cfg.LP)                # perm position -> prob
        out[c * cfg.L:(c + 1) * cfg.L, 0] = pl[invs[c]]
    return out
